# revision 1
# baseline (speedup 1.0000x reference)
"""Trainium2 Bass kernel for nn_Node3DEmbeddingv2 (gnn_message_passing).

Strategy (8 NeuronCores, SPMD, data-parallel over flattened (batch, query-row)):
  - 1536 query rows split into 8 x 192 (batch-aligned: 4 cores per batch).
  - Per core, per 32-row group: pairwise deltas vs all 768 keys on DVE,
    squared, reduced over xyz via a block-replicating matmul -> d^2 (PSUM),
    ACT sqrt -> d (replicated 3x across partitions).
  - d is split into 3 bf16 components (hi/mid/lo, exact to f32 precision);
    a [3,128]-ones bf16 matmul broadcasts each row's 768 distances across
    all 128 gaussian-channel partitions at full PE rate (f32 accumulate).
  - One ScalarE op per row computes the whole Gaussian:
      Derivative_Erf(scale_k * d + bias_k) = 2/sqrt(pi) * exp(-((d-m_k)/s_k)^2/2)
    with accum_out producing the sum over the 768 keys for free.
    (Fallback path: Square + Exp, two ACT passes, if the table is inaccurate.)
  - Channel constants 1/(sqrt(2 pi) s_k) (and the table constant) are applied
    post-reduction on the tiny [128, 192] summed tensor.
  - feature_proj MLP (gelu between two matmuls) on device; PE transposes the
    [E, rows] result back to row-major, adds the host-computed angle/time
    embedding tail, DMAs out [192, 512] per core.
  - Host (numpy, negligible): angle MLP, sinusoidal time embedding MLP,
    masking, per-core input prep; all heavy compute is on-device.
"""

import math

import numpy as np

# Problem constants (hardcoded per the task contract).
B, N, K, E = 2, 768, 128, 512
INTER = E // 2
NCORES = 8
RPC = (B * N) // NCORES  # 192 rows per core
GR = 32                  # rows per group
NGROUPS = RPC // GR      # 6 groups per core
PI_REF = 3.14159         # matches reference's gaussian constant

# Derivative_Erf table semantics: d/dx erf(x) = 2/sqrt(pi) * exp(-x^2).
# DERF_INV is the factor that converts the table output back to exp(-x^2).
DERF_INV = math.sqrt(math.pi) / 2.0

# Set to True to use the Square+Exp fallback instead of Derivative_Erf.
USE_FALLBACK_EXP = False
# Simulator/testing hook: replace Derivative_Erf by another func (e.g. Square).
_FUNC_OVERRIDE = None

_COMPILED = {}


def _enable_ldw_opt():
    """Flip walrus's redundant-LDWEIGHTS elimination on: our 384 broadcast
    matmuls reuse one stationary [3,128] ones matrix, and the per-matmul
    reload serializes ~134ns each on the PE. Correctness is re-verified
    end-to-end against the oracle after any compile-flag change."""
    from concourse import bass_utils

    if getattr(bass_utils, "_ldw_opt_patched", False):
        return
    orig = bass_utils.bir_verify_and_optimise

    def patched(*args, **kwargs):
        import subprocess

        orig_run = bass_utils.run_command

        def run_patched(argv, **kw):
            argv = [
                a.replace("--enable-ldw-opt=false", "--enable-ldw-opt=true")
                if isinstance(a, str) else a
                for a in argv
            ]
            return orig_run(argv, **kw)

        bass_utils.run_command = run_patched
        try:
            return orig(*args, **kwargs)
        finally:
            bass_utils.run_command = orig_run

    bass_utils.bir_verify_and_optimise = patched
    bass_utils._ldw_opt_patched = True


def _build_nc(use_fallback=None, func_override=None, gelu_override=None):
    import concourse.bass as bass
    import concourse.bacc as bacc
    from concourse import mybir
    from concourse.tile import TileContext

    # note: _enable_ldw_opt() breaks walrus codegen (standalone InstLdweights
    # with f32 matmuls in the module) — left available but unused

    if use_fallback is None:
        use_fallback = USE_FALLBACK_EXP
    f32 = mybir.dt.float32
    bf16 = mybir.dt.bfloat16
    AF = mybir.ActivationFunctionType

    nc = bacc.Bacc("TRN2", target_bir_lowering=False)

    # DRAM I/O (per-core values supplied via in_maps).
    posT = nc.dram_tensor("posT", [3, N], f32, kind="ExternalInput")
    qscal = nc.dram_tensor("qscal", [96, NGROUPS], f32, kind="ExternalInput")
    blk3 = nc.dram_tensor("blk3", [96, GR], f32, kind="ExternalInput")
    esc = nc.dram_tensor("esc", [K, 1], f32, kind="ExternalInput")
    ebi = nc.dram_tensor("ebi", [K, 1], f32, kind="ExternalInput")
    postc = nc.dram_tensor("postc", [K, 1], f32, kind="ExternalInput")
    w1 = nc.dram_tensor("w1", [K, K], f32, kind="ExternalInput")
    w2 = nc.dram_tensor("w2", [K, INTER], f32, kind="ExternalInput")
    ident = nc.dram_tensor("ident", [128, 128], f32, kind="ExternalInput")
    rest = nc.dram_tensor("rest", [RPC, E], f32, kind="ExternalInput")
    out = nc.dram_tensor("out", [RPC, E], f32, kind="ExternalOutput")

    with TileContext(nc) as tc:
        with tc.tile_pool(name="sb", bufs=1) as sb:
            # ---- constant loads ----
            pos_rep = sb.tile([96, N], f32, tag="pos_rep")
            nc.sync.dma_start(
                out=pos_rep,
                in_=bass.AP(tensor=posT, offset=0, ap=[[0, 32], [N, 3], [1, N]]),
            )
            q_sb = sb.tile([96, NGROUPS], f32, tag="q_sb")
            nc.sync.dma_start(out=q_sb, in_=qscal[:, :])
            blk_sb = sb.tile([96, GR], f32, tag="blk_sb")
            nc.sync.dma_start(out=blk_sb, in_=blk3[:, :])
            esc_sb = sb.tile([K, 1], f32, tag="esc_sb")
            nc.sync.dma_start(out=esc_sb, in_=esc[:, :])
            ebi_sb = sb.tile([K, 1], f32, tag="ebi_sb")
            nc.sync.dma_start(out=ebi_sb, in_=ebi[:, :])
            postc_sb = sb.tile([K, 1], f32, tag="postc_sb")
            nc.sync.dma_start(out=postc_sb, in_=postc[:, :])
            w1_sb = sb.tile([K, K], f32, tag="w1_sb")
            nc.sync.dma_start(out=w1_sb, in_=w1[:, :])
            w2_sb = sb.tile([K, INTER], f32, tag="w2_sb")
            nc.sync.dma_start(out=w2_sb, in_=w2[:, :])
            id_sb = sb.tile([128, 128], f32, tag="id_sb")
            nc.sync.dma_start(out=id_sb, in_=ident[:, :])
            ones3 = sb.tile([3, 128], bf16, tag="ones3")
            nc.vector.memset(ones3, 1.0)

            S = sb.tile([K, RPC], f32, tag="S")

            # Collapse the many input-DMA queue semaphores into one point so
            # downstream consumers never need more waits than the instruction
            # encoding allows.
            tc.strict_bb_all_engine_barrier()

            # ---- phase A: distances + bf16 splits for all 6 groups ----
            split_tiles = []
            with tc.tile_pool(name="psA", bufs=1, space="PSUM") as psA:
                for g in range(NGROUPS):
                    delta = sb.tile([96, N], f32, tag="delta", bufs=2)
                    nc.vector.tensor_scalar(
                        out=delta,
                        in0=pos_rep,
                        scalar1=q_sb[:, g : g + 1],
                        scalar2=None,
                        op0=mybir.AluOpType.subtract,
                    )
                    nc.vector.tensor_mul(delta, delta, delta)
                    psum_d2 = psA.tile([GR, N], f32, tag="d2", bufs=2)
                    nc.tensor.matmul(
                        psum_d2[:, 0:512], blk_sb, delta[:, 0:512],
                        start=True, stop=True,
                    )
                    nc.tensor.matmul(
                        psum_d2[:, 512:N], blk_sb, delta[:, 512:N],
                        start=True, stop=True,
                    )
                    d_sb = sb.tile([GR, N], f32, tag=f"d{g}")
                    nc.scalar.sqrt(d_sb, psum_d2)
                    # exact 3-way bf16 split: hi + mid + lo == d (f32 precision)
                    dh = sb.tile([GR, N], bf16, tag=f"dh{g}")
                    nc.vector.tensor_copy(dh, d_sb)
                    r1 = sb.tile([GR, N], f32, tag="r1", bufs=2)
                    nc.vector.tensor_sub(r1, d_sb, dh)
                    dm = sb.tile([GR, N], bf16, tag=f"dm{g}")
                    nc.vector.tensor_copy(dm, r1)
                    r2 = sb.tile([GR, N], f32, tag="r2", bufs=2)
                    nc.vector.tensor_sub(r2, r1, dm)
                    dl = sb.tile([GR, N], bf16, tag=f"dl{g}")
                    nc.vector.tensor_copy(dl, r2)
                    split_tiles.append((dh, dm, dl))

            # ---- phase B: broadcast + gaussian + key-sum per row ----
            derf_func = AF.Derivative_Erf
            if func_override is not None:
                derf_func = func_override
            with tc.tile_pool(name="psB", bufs=1, space="PSUM") as psB:
                ones2q = sb.tile([67, 128], bf16, tag="ones2q")
                nc.vector.memset(ones2q, 1.0)
                SG = GR // 2
                for g2 in range(NGROUPS * 2):
                    g, s = divmod(g2, 2)
                    # flatten 16 rows onto two PE quadrant trios (partitions
                    # 0..2 and 64..66): 8 rows each. Alternating matmul issue
                    # between the quadrants lets the PE pipeline them
                    # concurrently (~2x effective rate).
                    M_flat = sb.tile([67, SG * N // 2], bf16, tag="mflat", bufs=2)
                    for c, comp in enumerate(split_tiles[g]):
                        for q in range(2):
                            nc.gpsimd.dma_start(
                                out=M_flat[
                                    64 * q + c : 64 * q + c + 1, :
                                ].rearrange("p (a j) -> p a j", a=SG // 2),
                                in_=comp[
                                    SG * s + 8 * q : SG * s + 8 * (q + 1), :
                                ],
                            )
                    for a0 in range(0, SG, 4):
                        a = SG * s + a0
                        r = g * GR + a
                        unit = r // 4
                        # every 8th 4-row unit sums on the ScalarE accumulator
                        # (1-row activations with accum_out) to offload the DVE
                        act_accum_unit = (not use_fallback) and unit % 8 == 7
                        # 4-row macro unit: two 2-row PSUM tiles -> one 4-row
                        # gsc tile. Rows a0, a0+1 come from quadrant 0
                        # (partitions 0..2), rows a0+2, a0+3 (= slot a0, a0+1
                        # of the upper half) from quadrant 2 (partitions
                        # 64..66); issue alternates between the two so their
                        # matmuls overlap in the PE array.
                        gsc = sb.tile([K, 4, N], f32, tag="gsc", bufs=4)
                        mms = []
                        psums = []
                        base = (a0 // 4) * 2 * N  # slot pair 2u, 2u+1
                        for h in range(2):
                            psum_db = psB.tile([K, 2, N], f32, tag="db", bufs=2)
                            flat = psum_db.rearrange("k a j -> k (a j)")
                            qb = 64 * h
                            for lo in (0, 512, 1024):
                                mms.append(
                                    (
                                        flat[:, lo : lo + 512],
                                        ones2q[qb : qb + 3, :],
                                        M_flat[qb : qb + 3, base + lo : base + lo + 512],
                                        (qb, 0),
                                    )
                                )
                            psums.append(psum_db)
                        for idx in (0, 3, 1, 4, 2, 5):
                            out_ap, lhsT, rhs, tp = mms[idx]
                            nc.tensor.matmul(
                                out_ap, lhsT, rhs,
                                start=True, stop=True, tile_position=tp,
                            )
                        for h in range(2):
                            psum_db = psums[h]
                            if act_accum_unit:
                                for q in range(2):
                                    nc.scalar.activation(
                                        out=gsc[:, 2 * h + q, :],
                                        in_=psum_db[:, q, :],
                                        func=derf_func,
                                        bias=ebi_sb,
                                        scale=esc_sb,
                                        accum_out=S[:, r + 2 * h + q : r + 2 * h + q + 1],
                                    )
                            elif not use_fallback:
                                nc.scalar.activation(
                                    out=gsc[:, 2 * h : 2 * h + 2, :],
                                    in_=psum_db,
                                    func=derf_func,
                                    bias=ebi_sb,
                                    scale=esc_sb,
                                )
                            else:
                                zsq = sb.tile([K, 2, N], f32, tag="zsq", bufs=3)
                                nc.scalar.activation(
                                    out=zsq, in_=psum_db,
                                    func=AF.Square, bias=ebi_sb, scale=esc_sb,
                                )
                                nc.scalar.activation(
                                    out=gsc[:, 2 * h : 2 * h + 2, :], in_=zsq,
                                    func=AF.Exp, bias=postc_sb, scale=-0.5,
                                )
                        if not act_accum_unit:
                            # key-axis sum on DVE (4 rows per op)
                            nc.vector.reduce_sum(
                                out=S[:, r : r + 4], in_=gsc,
                                axis=mybir.AxisListType.X,
                            )

            # ---- phase C: channel constants + feature_proj MLP + output ----
            # processed in two 96-row chunks so the second half of phase B can
            # still be running while the first chunk's MLP drains
            with tc.tile_pool(name="psC", bufs=1, space="PSUM") as psC:
                gelu_func = AF.Gelu if gelu_override is None else gelu_override
                for t in range(2):
                    rows = slice(96 * t, 96 * (t + 1))
                    if not use_fallback:
                        nc.vector.tensor_scalar_mul(
                            S[:, rows], S[:, rows], postc_sb
                        )
                    psum_h = psC.tile([K, 96], f32, tag="mlp", bufs=2)
                    nc.tensor.matmul(psum_h, w1_sb, S[:, rows], start=True, stop=True)
                    h_sb = sb.tile([K, 96], f32, tag="h_sb", bufs=2)
                    nc.scalar.activation(h_sb, psum_h, gelu_func)
                    o_sb = sb.tile([128, 2, 96], f32, tag="o_sb", bufs=2)
                    for e in range(2):
                        psum_o = psC.tile([128, 96], f32, tag="mlp", bufs=2)
                        nc.tensor.matmul(
                            psum_o, w2_sb[:, 128 * e : 128 * (e + 1)], h_sb,
                            start=True, stop=True,
                        )
                        nc.vector.tensor_copy(o_sb[:, e, :], psum_o)
                    out_sb = sb.tile([96, E], f32, tag=f"out{t}")
                    nc.gpsimd.dma_start(
                        out=out_sb, in_=rest[96 * t : 96 * (t + 1), :]
                    )
                    for e in range(2):
                        psum_t = psC.tile([96, 128], f32, tag="tr", bufs=2)
                        nc.tensor.transpose(psum_t, o_sb[:, e, :], id_sb)
                        nc.vector.tensor_add(
                            out_sb[:, 128 * e : 128 * (e + 1)],
                            out_sb[:, 128 * e : 128 * (e + 1)],
                            psum_t,
                        )
                    nc.sync.dma_start(
                        out=out[96 * t : 96 * (t + 1), :], in_=out_sb
                    )

    nc.compile()
    return nc


# ---------------- host-side reference tails (numpy, f32) ----------------

def _erf_np(x):
    try:
        from scipy.special import erf
        return erf(x).astype(np.float32)
    except ImportError:
        f = np.frompyfunc(math.erf, 1, 1)
        return f(x.astype(np.float64)).astype(np.float32)


def _gelu_np(x):
    x = x.astype(np.float32)
    return (x * 0.5 * (1.0 + _erf_np(x / np.float32(math.sqrt(2.0))))).astype(
        np.float32
    )


def _silu_np(x):
    x = x.astype(np.float32)
    return (x / (1.0 + np.exp(-x))).astype(np.float32)


def _timestep_emb_np(t, dim):
    half = dim // 2
    freqs = np.exp(
        -np.log(10000.0) * np.arange(half, dtype=np.float32) / np.float32(half)
    ).astype(np.float32)
    a = t.astype(np.float32)[:, None] * freqs[None, :]
    return np.concatenate([np.sin(a), np.cos(a)], axis=-1).astype(np.float32)


def _host_tails(angle, mask_pos, time_pos, ang_w1, ang_w2, t_w1, t_b1, t_w2, t_b2):
    """rest[b, n, :] with rest[..., :INTER] = time_emb[..., :INTER] and
    rest[..., INTER:] = ang_f + time_emb[..., INTER:]."""
    angle = np.asarray(angle, np.float32)
    ang = np.where(np.isposinf(angle), np.float32(0.0), angle).astype(np.float32)
    ang_f = _gelu_np(ang @ np.asarray(ang_w1, np.float32)) @ np.asarray(
        ang_w2, np.float32
    )  # [B, N, INTER]

    def time_mlp(t):
        e = _timestep_emb_np(t, E)
        h = _silu_np(e @ np.asarray(t_w1, np.float32) + np.asarray(t_b1, np.float32))
        return (h @ np.asarray(t_w2, np.float32) + np.asarray(t_b2, np.float32)).astype(
            np.float32
        )

    tp = np.asarray(time_pos)
    te = time_mlp(tp)[:, None, :]                 # [B, 1, E]
    t0e = time_mlp(np.zeros_like(tp))[:, None, :]
    mask = np.asarray(mask_pos, bool)             # [B, N, 1]
    time_emb = np.where(mask, te, t0e).astype(np.float32)  # [B, N, E]

    rest = time_emb.copy()
    rest[..., INTER:] += ang_f.astype(np.float32)
    return rest.astype(np.float32)


def _prep_in_maps(pos, angle, padding_mask, mask_pos, time_pos,
                  means, stds, fp_w1, fp_w2, ang_w1, ang_w2,
                  t_w1, t_b1, t_w2, t_b2, use_fallback=None):
    if use_fallback is None:
        use_fallback = USE_FALLBACK_EXP
    pos = np.asarray(pos, np.float32)
    pad = np.asarray(padding_mask, bool)

    s = (np.abs(np.asarray(stds, np.float32)) + np.float32(0.01)).astype(np.float32)
    m = np.asarray(means, np.float32)
    inv_s = (np.float32(1.0) / s).astype(np.float32)
    if not use_fallback:
        # Derivative_Erf(x) with x = (d - m)/(s*sqrt(2))
        esc_v = (inv_s / np.float32(math.sqrt(2.0))).astype(np.float32)
        ebi_v = (-m * esc_v).astype(np.float32)
        postc_v = (
            np.float32(DERF_INV) / (np.float32(math.sqrt(2.0 * PI_REF)) * s)
        ).astype(np.float32)
    else:
        # Square then Exp(-0.5 z^2 + log c)
        esc_v = inv_s.astype(np.float32)
        ebi_v = (-m * inv_s).astype(np.float32)
        postc_v = np.log(
            np.float32(1.0) / (np.float32(math.sqrt(2.0 * PI_REF)) * s)
        ).astype(np.float32)

    blk3 = np.zeros((96, GR), np.float32)
    for p in range(96):
        blk3[p, p // 3] = 1.0

    rest = _host_tails(
        angle, mask_pos, time_pos, ang_w1, ang_w2, t_w1, t_b1, t_w2, t_b2
    )

    ident = np.eye(128, dtype=np.float32)
    w1_v = np.asarray(fp_w1, np.float32)
    w2_v = np.asarray(fp_w2, np.float32)

    in_maps = []
    for c in range(NCORES):
        b = c // (NCORES // B)
        r0 = (c % (NCORES // B)) * RPC
        posT = pos[b].T.copy()  # [3, N]
        if pad[b].any():
            posT[:, pad[b]] = np.float32(1.0e6)
        # phase-A partition rr holds the query row that lands on PE quadrant
        # 0 (first 8 of each 16-row subgroup) or quadrant 2 (last 8), so the
        # M_flat flatten DMAs stay partition-contiguous while consecutive
        # device rows alternate quadrants (rows a0,a0+1 -> Q0; a0+2,a0+3 -> Q2)
        perm16 = np.array([0, 1, 4, 5, 8, 9, 12, 13, 2, 3, 6, 7, 10, 11, 14, 15])
        perm = np.concatenate([perm16, 16 + perm16])
        qscal = np.empty((96, NGROUPS), np.float32)
        for g in range(NGROUPS):
            rows = pos[b, r0 + g * GR : r0 + (g + 1) * GR, :][perm]  # [32, 3]
            qscal[:, g] = rows.reshape(-1)
        in_maps.append(
            {
                "posT": np.ascontiguousarray(posT, np.float32),
                "qscal": qscal,
                "blk3": blk3,
                "esc": esc_v.reshape(K, 1),
                "ebi": ebi_v.reshape(K, 1),
                "postc": postc_v.reshape(K, 1),
                "w1": w1_v,
                "w2": w2_v,
                "ident": ident,
                "rest": np.ascontiguousarray(rest[b, r0 : r0 + RPC, :], np.float32),
            }
        )
    return in_maps


def kernel(pos, angle, node_type_edge, padding_mask, mask_aa, mask_pos, time_pos,
           means, stds, fp_w1, fp_w2, ang_w1, ang_w2, t_w1, t_b1, t_w2, t_b2):
    from concourse.bass_utils import run_bass_kernel_spmd

    key = ("nc", USE_FALLBACK_EXP, _FUNC_OVERRIDE)
    if key not in _COMPILED:
        _COMPILED[key] = _build_nc(func_override=_FUNC_OVERRIDE)
    nc = _COMPILED[key]

    in_maps = _prep_in_maps(
        pos, angle, padding_mask, mask_pos, time_pos, means, stds,
        fp_w1, fp_w2, ang_w1, ang_w2, t_w1, t_b1, t_w2, t_b2,
    )
    res = run_bass_kernel_spmd(nc, in_maps, core_ids=list(range(NCORES)))
    outs = [np.asarray(res.results[c]["out"], np.float32) for c in range(NCORES)]
    full = np.concatenate(outs, axis=0).reshape(B, N, E)
    return full



# revision 4
# speedup vs baseline: 2.6859x; 2.6859x over previous
"""Trainium2 Bass kernel for nn_Node3DEmbeddingv2 (gnn_message_passing).

Strategy (8 NeuronCores, SPMD, data-parallel over flattened (batch, query-row)):
  - 1536 query rows split 8 x 192 (4 cores per batch). Per core, rows live on
    SBUF partitions: d1 [128 rows, 768 keys] plus a packed d2 [128, 384]
    holding rows 128..191 as (row, key-half) slots, so all 128 lanes stay hot.
  - Pairwise distances via one 5-term Gram matmul per block:
      lhsT cols (-2x,-2y,-2z,|p|^2,1) x rhs rows (x,y,z,1,|p|^2) -> d^2 in
    PSUM; DVE clamps tiny negative round-off, one Sqrt ACT -> d.
  - The 128 gaussian channels are compressed onto C=32 gaussian atoms
    exp(-(a_c d + b_c)^2) fitted at runtime (host, numpy Levenberg-Marquardt
    on a density-weighted quadrature of the empirical distance distribution;
    the objective penalizes density-weighted bias so the 768-key row sums
    don't accumulate systematic fit error). Each atom is one Derivative_Erf
    ACT pass over d1/d2 with per-partition scale/bias vectors (runtime data,
    no recompile), bf16 out; DVE reduce_sum over keys -> per-row atom sums.
  - The [C -> K] mixing matrix is folded into fp_w1 on host (w1p = A @ fp_w1),
    so the feature_proj MLP runs unchanged: gelu(sum_basis @ w1p) @ fp_w2,
    with PE transposes back to row-major and the host-computed angle/time
    embedding tail added before DMA out ([192, 512] per core).
  - Host (numpy, negligible vs HW): atom fit (cached), angle MLP, sinusoidal
    time MLP, per-core input prep.
"""

import hashlib
import math

import numpy as np

# Problem constants (hardcoded per the task contract).
B, N, K, E = 2, 768, 128, 512
INTER = E // 2
NCORES = 8
RPC = (B * N) // NCORES  # 192 rows per core
C = 32                   # gaussian atoms after compression
PI_REF = 3.14159         # matches reference's gaussian constant
SQ2 = math.sqrt(2.0)
# Derivative_Erf(x) = 2/sqrt(pi) * exp(-x^2); DERF_INV converts back.
DERF_INV = math.sqrt(math.pi) / 2.0

# compat knobs referenced by test.py (unused by this implementation)
USE_FALLBACK_EXP = False
_FUNC_OVERRIDE = None

_COMPILED = {}
_FIT_CACHE = {}


# ---------------- runtime atom fit (numpy-only) ----------------

def _fit_atoms(means, s, xs, rho_row, n_atoms, iters=150, seed_thresh=0.25, seed=0):
    """Fit f_k(d) ~= sum_c A[c,k] exp(-(a_c d + b_c)^2).

    Weighted rows: sqrt(rho) pointwise (variance of the 768-key row sum) plus
    one density-sum row (bias of the row sum). VarPro: A by ridge lstsq each
    step, Levenberg-Marquardt on (a, b)."""
    rng = np.random.default_rng(seed)
    Kn = len(means)
    F = np.exp(-0.5 * ((xs[:, None] - means) / s) ** 2) / (np.sqrt(2 * PI_REF) * s)
    wvar = np.sqrt(rho_row)

    def weight_rows(M):
        return np.concatenate(
            [M * wvar[:, None], (M * rho_row[:, None]).sum(0, keepdims=True)], 0
        )

    Fw = weight_rows(F)
    a = np.empty(n_atoms)
    b = np.empty(n_atoms)
    idx = 0
    for k in np.argsort(s):
        if s[k] < seed_thresh and idx < n_atoms:
            a[idx] = 1.0 / (SQ2 * s[k])
            b[idx] = -means[k] / (SQ2 * s[k])
            idx += 1
    nrem = n_atoms - idx
    if nrem > 0:
        sig_levels = np.geomspace(0.25, 6.0, nrem)
        mus = np.interp(
            np.linspace(0, 1, nrem), np.linspace(0, 1, Kn), np.sort(means)
        )
        rng.shuffle(mus)
        for i in range(nrem):
            a[idx] = 1.0 / sig_levels[i]
            b[idx] = -(mus[i] + rng.normal(0, 0.2)) / sig_levels[i]
            idx += 1

    def atoms_of(a, b):
        Z = a[None, :] * xs[:, None] + b[None, :]
        return Z, np.exp(-(Z ** 2))

    def solve_A(Gw):
        M = Gw.T @ Gw
        M = M + 1e-9 * np.eye(n_atoms) * np.trace(M) / n_atoms
        return np.linalg.solve(M, Gw.T @ Fw)

    lam = 1e-3
    Z, G = atoms_of(a, b)
    Gw = weight_rows(G)
    A = solve_A(Gw)
    R = Fw - Gw @ A
    err = np.linalg.norm(R)
    for _ in range(iters):
        Da = -2 * Z * xs[:, None] * G
        Db = -2 * Z * G
        Daw = weight_rows(Da)
        Dbw = weight_rows(Db)
        AA = A @ A.T
        RA = R @ A.T
        n2 = 2 * n_atoms
        JtJ = np.empty((n2, n2))
        JtJ[:n_atoms, :n_atoms] = AA * (Daw.T @ Daw)
        JtJ[:n_atoms, n_atoms:] = AA * (Daw.T @ Dbw)
        JtJ[n_atoms:, :n_atoms] = JtJ[:n_atoms, n_atoms:].T
        JtJ[n_atoms:, n_atoms:] = AA * (Dbw.T @ Dbw)
        Jtr = np.concatenate([-np.sum(Daw * RA, 0), -np.sum(Dbw * RA, 0)])
        ok = False
        for _ in range(8):
            try:
                step = np.linalg.solve(
                    JtJ + lam * np.diag(np.diag(JtJ)) + 1e-12 * np.eye(n2), -Jtr
                )
            except np.linalg.LinAlgError:
                lam *= 10
                continue
            a2 = a + step[:n_atoms]
            b2 = b + step[n_atoms:]
            Z2, G2 = atoms_of(a2, b2)
            Gw2 = weight_rows(G2)
            A2 = solve_A(Gw2)
            R2 = Fw - Gw2 @ A2
            e2 = np.linalg.norm(R2)
            if e2 < err:
                a, b, Z, G, Gw, A, R, err = a2, b2, Z2, G2, Gw2, A2, R2, e2
                lam = max(lam * 0.5, 1e-7)
                ok = True
                break
            lam *= 4
        if not ok:
            break
    return a, b, A, err / np.linalg.norm(Fw)


def _fit_atoms_best(means, stds, dist_samples):
    """Multi-restart fit keyed on the inputs; returns (a, b, A)."""
    key = hashlib.sha1(
        means.tobytes() + stds.tobytes() + dist_samples.tobytes()
    ).hexdigest()
    if key in _FIT_CACHE:
        return _FIT_CACHE[key]
    s = (np.abs(means * 0) + np.abs(stds) + 0.01).astype(np.float64)
    means64 = means.astype(np.float64)
    xs = np.concatenate([np.arange(0.0, 6.0, 0.01), np.arange(6.0, 40.0, 0.04)])
    hist, _ = np.histogram(dist_samples, bins=np.concatenate([xs, [40.0]]))
    nrows = max(len(dist_samples) // N, 1)
    rho_row = hist.astype(np.float64) / nrows
    best = None
    for trial in range(3):
        a, b, A, rel = _fit_atoms(means64, s, xs, rho_row, C, seed=trial)
        if best is None or rel < best[3]:
            best = (a, b, A, rel)
        if rel < 1.5e-4:
            break
    a, b, A, rel = best
    _FIT_CACHE[key] = (a, b, A)
    return a, b, A


# ---------------- device program ----------------

def _build_nc(use_fallback=None, func_override=None, gelu_override=None):
    import concourse.bass as bass  # noqa: F401
    import concourse.bacc as bacc
    from concourse import mybir
    from concourse.tile import TileContext

    f32 = mybir.dt.float32
    bf16 = mybir.dt.bfloat16
    AF = mybir.ActivationFunctionType
    X = mybir.AxisListType.X

    derf_func = AF.Derivative_Erf if func_override is None else func_override
    gelu_func = AF.Gelu if gelu_override is None else gelu_override

    nc = bacc.Bacc("TRN2", target_bir_lowering=False)

    l5a = nc.dram_tensor("l5a", [5, 128], f32, kind="ExternalInput")
    l5b = nc.dram_tensor("l5b", [5, 64], f32, kind="ExternalInput")
    r5 = nc.dram_tensor("r5", [5, N], f32, kind="ExternalInput")
    scl = nc.dram_tensor("scl", [128, C], f32, kind="ExternalInput")
    bia = nc.dram_tensor("bia", [128, C], f32, kind="ExternalInput")
    w1p = nc.dram_tensor("w1p", [C, K], f32, kind="ExternalInput")
    w2 = nc.dram_tensor("w2", [K, INTER], f32, kind="ExternalInput")
    ident = nc.dram_tensor("ident", [128, 128], f32, kind="ExternalInput")
    rest = nc.dram_tensor("rest", [RPC, E], f32, kind="ExternalInput")
    out = nc.dram_tensor("out", [RPC, E], f32, kind="ExternalOutput")

    with TileContext(nc) as tc:
        with tc.tile_pool(name="sb", bufs=1) as sb:
            # ---- constant loads ----
            l5a_sb = sb.tile([5, 128], f32, tag="l5a")
            nc.sync.dma_start(out=l5a_sb, in_=l5a[:, :])
            l5b_sb = sb.tile([5, 64], f32, tag="l5b")
            nc.sync.dma_start(out=l5b_sb, in_=l5b[:, :])
            r5_sb = sb.tile([5, N], f32, tag="r5")
            nc.sync.dma_start(out=r5_sb, in_=r5[:, :])
            scl_sb = sb.tile([128, C], f32, tag="scl")
            nc.sync.dma_start(out=scl_sb, in_=scl[:, :])
            bia_sb = sb.tile([128, C], f32, tag="bia")
            nc.sync.dma_start(out=bia_sb, in_=bia[:, :])
            w1p_sb = sb.tile([C, K], f32, tag="w1p")
            nc.sync.dma_start(out=w1p_sb, in_=w1p[:, :])
            w2_sb = sb.tile([K, INTER], f32, tag="w2")
            nc.sync.dma_start(out=w2_sb, in_=w2[:, :])
            id_sb = sb.tile([128, 128], f32, tag="id")
            nc.sync.dma_start(out=id_sb, in_=ident[:, :])
            out_sbs = []
            for t in range(2):
                o = sb.tile([96, E], f32, tag=f"out{t}")
                nc.gpsimd.dma_start(out=o, in_=rest[96 * t : 96 * (t + 1), :])
                out_sbs.append(o)

            d1_sb = sb.tile([128, N], f32, tag="d1")
            d2_sb = sb.tile([128, N // 2], f32, tag="d2")
            acc1 = sb.tile([128, C], f32, tag="acc1")
            acc2 = sb.tile([128, C], f32, tag="acc2")

            tc.strict_bb_all_engine_barrier()

            # ---- phase A: pairwise distances ----
            with tc.tile_pool(name="psA", bufs=1, space="PSUM") as psA:
                ps_a = psA.tile([128, N], f32, tag="d2a")
                nc.tensor.matmul(ps_a[:, 0:512], l5a_sb, r5_sb[:, 0:512],
                                 start=True, stop=True)
                nc.tensor.matmul(ps_a[:, 512:N], l5a_sb, r5_sb[:, 512:N],
                                 start=True, stop=True)
                ps_b = psA.tile([128, N // 2], f32, tag="d2b")
                nc.tensor.matmul(ps_b[0:64, :], l5b_sb, r5_sb[:, 0:384],
                                 start=True, stop=True, tile_position=(0, 0))
                nc.tensor.matmul(ps_b[64:128, :], l5b_sb, r5_sb[:, 384:N],
                                 start=True, stop=True, tile_position=(0, 64))
                # clamp f32 round-off (gram form can go slightly negative)
                nc.vector.tensor_scalar_max(ps_a, ps_a, 0.0)
                nc.vector.tensor_scalar_max(ps_b, ps_b, 0.0)
                nc.scalar.sqrt(d1_sb, ps_a)
                nc.scalar.sqrt(d2_sb, ps_b)

            # ---- phase B: atom ACT passes + key-axis reduction ----
            for c in range(C):
                g1 = sb.tile([128, N], bf16, tag="g1", bufs=3)
                nc.scalar.activation(
                    out=g1, in_=d1_sb, func=derf_func,
                    bias=bia_sb[:, c : c + 1], scale=scl_sb[:, c : c + 1],
                )
                g2 = sb.tile([128, N // 2], bf16, tag="g2", bufs=3)
                nc.scalar.activation(
                    out=g2, in_=d2_sb, func=derf_func,
                    bias=bia_sb[:, c : c + 1], scale=scl_sb[:, c : c + 1],
                )
                nc.vector.reduce_sum(out=acc1[:, c : c + 1], in_=g1, axis=X)
                nc.vector.reduce_sum(out=acc2[:, c : c + 1], in_=g2, axis=X)

            # ---- phase C: MLP + transpose + tail add + store ----
            with tc.tile_pool(name="psC", bufs=1, space="PSUM") as psC:
                t1 = psC.tile([C, 128], f32, tag="t1")
                nc.tensor.transpose(t1, acc1, id_sb)
                t2 = psC.tile([C, 128], f32, tag="t2")
                nc.tensor.transpose(t2, acc2, id_sb)
                sbm = sb.tile([C, RPC], f32, tag="sbm")
                nc.vector.tensor_copy(sbm[:, 0:128], t1)
                nc.vector.tensor_copy(sbm[:, 128:RPC], t2[:, 0:64])
                nc.vector.tensor_add(sbm[:, 128:RPC], sbm[:, 128:RPC], t2[:, 64:128])

                ps_h = psC.tile([K, RPC], f32, tag="h")
                nc.tensor.matmul(ps_h, w1p_sb, sbm, start=True, stop=True)
                h_sb = sb.tile([K, RPC], f32, tag="h_sb")
                nc.scalar.activation(h_sb, ps_h, gelu_func)

                o_sb = sb.tile([128, 2, RPC], f32, tag="o_sb")
                for e in range(2):
                    ps_o = psC.tile([128, RPC], f32, tag="o", bufs=2)
                    nc.tensor.matmul(
                        ps_o, w2_sb[:, 128 * e : 128 * (e + 1)], h_sb,
                        start=True, stop=True,
                    )
                    nc.vector.tensor_copy(o_sb[:, e, :], ps_o)
                for t in range(2):
                    out_sb = out_sbs[t]
                    for e in range(2):
                        ps_t = psC.tile([96, 128], f32, tag="tr", bufs=2)
                        nc.tensor.transpose(
                            ps_t, o_sb[:, e, 96 * t : 96 * (t + 1)], id_sb
                        )
                        nc.vector.tensor_add(
                            out_sb[:, 128 * e : 128 * (e + 1)],
                            out_sb[:, 128 * e : 128 * (e + 1)],
                            ps_t,
                        )
                    nc.sync.dma_start(
                        out=out[96 * t : 96 * (t + 1), :], in_=out_sb
                    )

    nc.compile()
    return nc


# ---------------- host-side reference tails (numpy, f32) ----------------

def _erf_np(x):
    try:
        from scipy.special import erf
        return erf(x).astype(np.float32)
    except ImportError:
        f = np.frompyfunc(math.erf, 1, 1)
        return f(x.astype(np.float64)).astype(np.float32)


def _gelu_np(x):
    x = x.astype(np.float32)
    return (x * 0.5 * (1.0 + _erf_np(x / np.float32(math.sqrt(2.0))))).astype(
        np.float32
    )


def _silu_np(x):
    x = x.astype(np.float32)
    return (x / (1.0 + np.exp(-x))).astype(np.float32)


def _timestep_emb_np(t, dim):
    half = dim // 2
    freqs = np.exp(
        -np.log(10000.0) * np.arange(half, dtype=np.float32) / np.float32(half)
    ).astype(np.float32)
    a = t.astype(np.float32)[:, None] * freqs[None, :]
    return np.concatenate([np.sin(a), np.cos(a)], axis=-1).astype(np.float32)


def _host_tails(angle, mask_pos, time_pos, ang_w1, ang_w2, t_w1, t_b1, t_w2, t_b2):
    """rest[b, n, :] with rest[..., :INTER] = time_emb[..., :INTER] and
    rest[..., INTER:] = ang_f + time_emb[..., INTER:]."""
    angle = np.asarray(angle, np.float32)
    ang = np.where(np.isposinf(angle), np.float32(0.0), angle).astype(np.float32)
    ang_f = _gelu_np(ang @ np.asarray(ang_w1, np.float32)) @ np.asarray(
        ang_w2, np.float32
    )  # [B, N, INTER]

    def time_mlp(t):
        e = _timestep_emb_np(t, E)
        h = _silu_np(e @ np.asarray(t_w1, np.float32) + np.asarray(t_b1, np.float32))
        return (h @ np.asarray(t_w2, np.float32) + np.asarray(t_b2, np.float32)).astype(
            np.float32
        )

    tp = np.asarray(time_pos)
    te = time_mlp(tp)[:, None, :]                 # [B, 1, E]
    t0e = time_mlp(np.zeros_like(tp))[:, None, :]
    mask = np.asarray(mask_pos, bool)             # [B, N, 1]
    time_emb = np.where(mask, te, t0e).astype(np.float32)  # [B, N, E]

    rest = time_emb.copy()
    rest[..., INTER:] += ang_f.astype(np.float32)
    return rest.astype(np.float32)


def _prep_in_maps(pos, angle, padding_mask, mask_pos, time_pos,
                  means, stds, fp_w1, fp_w2, ang_w1, ang_w2,
                  t_w1, t_b1, t_w2, t_b2, use_fallback=None):
    pos = np.asarray(pos, np.float32)
    pad = np.asarray(padding_mask, bool)
    means = np.asarray(means, np.float32)
    stds = np.asarray(stds, np.float32)

    # distance samples for the fit density (valid keys only)
    pos64 = pos.astype(np.float64)
    d_samples = []
    for bb in range(B):
        dd = np.sqrt(
            np.maximum(
                ((pos64[bb][:, None, :] - pos64[bb][None, :, :]) ** 2).sum(-1), 0.0
            )
        )
        valid = ~pad[bb]
        d_samples.append(dd[:, valid].reshape(-1))
    d_samples = np.concatenate(d_samples)
    a_c, b_c, A = _fit_atoms_best(means, stds, d_samples)

    # fold Derivative_Erf's 2/sqrt(pi) and the mixing into fp_w1
    A_eff = (A * DERF_INV).astype(np.float64)  # [C, K]
    w1p_v = (A_eff @ np.asarray(fp_w1, np.float64)).astype(np.float32)  # [C, K]
    w2_v = np.asarray(fp_w2, np.float32)

    scl_v = np.broadcast_to(a_c.astype(np.float32), (128, C)).copy()
    bia_v = np.broadcast_to(b_c.astype(np.float32), (128, C)).copy()
    ident = np.eye(128, dtype=np.float32)

    rest = _host_tails(
        angle, mask_pos, time_pos, ang_w1, ang_w2, t_w1, t_b1, t_w2, t_b2
    )

    in_maps = []
    for core in range(NCORES):
        bb = core // (NCORES // B)
        r0 = (core % (NCORES // B)) * RPC
        p = pos[bb]  # [N, 3]
        n_all = (p.astype(np.float64) ** 2).sum(-1).astype(np.float32)  # [N]
        # rhs: keys
        r5_v = np.empty((5, N), np.float32)
        r5_v[0:3] = p.T
        r5_v[3] = 1.0
        r5_v[4] = n_all
        if pad[bb].any():
            r5_v[4, pad[bb]] = 1.0e12  # huge d^2 -> atoms vanish
        # lhsT: query rows (-2x,-2y,-2z,n,1)
        rows = p[r0 : r0 + RPC]  # [192, 3]
        nr = n_all[r0 : r0 + RPC]
        l5 = np.empty((5, RPC), np.float32)
        l5[0:3] = -2.0 * rows.T
        l5[3] = nr
        l5[4] = 1.0
        in_maps.append(
            {
                "l5a": np.ascontiguousarray(l5[:, 0:128]),
                "l5b": np.ascontiguousarray(l5[:, 128:RPC]),
                "r5": r5_v,
                "scl": scl_v,
                "bia": bia_v,
                "w1p": w1p_v,
                "w2": w2_v,
                "ident": ident,
                "rest": np.ascontiguousarray(rest[bb, r0 : r0 + RPC, :], np.float32),
            }
        )
    return in_maps


def kernel(pos, angle, node_type_edge, padding_mask, mask_aa, mask_pos, time_pos,
           means, stds, fp_w1, fp_w2, ang_w1, ang_w2, t_w1, t_b1, t_w2, t_b2):
    from concourse.bass_utils import run_bass_kernel_spmd

    key = ("nc", USE_FALLBACK_EXP, _FUNC_OVERRIDE)
    if key not in _COMPILED:
        _COMPILED[key] = _build_nc(func_override=_FUNC_OVERRIDE)
    nc = _COMPILED[key]

    in_maps = _prep_in_maps(
        pos, angle, padding_mask, mask_pos, time_pos, means, stds,
        fp_w1, fp_w2, ang_w1, ang_w2, t_w1, t_b1, t_w2, t_b2,
    )
    res = run_bass_kernel_spmd(nc, in_maps, core_ids=list(range(NCORES)))
    outs = [np.asarray(res.results[c]["out"], np.float32) for c in range(NCORES)]
    full = np.concatenate(outs, axis=0).reshape(B, N, E)
    return full


# revision 17
# speedup vs baseline: 3.0727x; 1.1440x over previous
"""Trainium2 Bass kernel for nn_Node3DEmbeddingv2 (gnn_message_passing).

Strategy (8 NeuronCores, SPMD, data-parallel over flattened (batch, query-row)):
  - 1536 query rows split 8 x 192 (4 cores per batch). Per core, rows live on
    SBUF partitions: d1 [128 rows, 768 keys] plus a packed d2 [128, 384]
    holding rows 128..191 as (row, key-half) slots, so all 128 lanes stay hot.
  - Pairwise distances via one 5-term Gram matmul per block:
      lhsT cols (-2x,-2y,-2z,|p|^2,1) x rhs rows (x,y,z,1,|p|^2) -> d^2 in
    PSUM; DVE clamps tiny negative round-off, one Sqrt ACT -> d.
  - The 128 gaussian channels are compressed onto C=32 gaussian atoms
    exp(-(a_c d + b_c)^2) fitted at runtime (host, numpy Levenberg-Marquardt
    on a density-weighted quadrature of the empirical distance distribution;
    the objective penalizes density-weighted bias so the 768-key row sums
    don't accumulate systematic fit error). Each atom is one Derivative_Erf
    ACT pass over d1/d2 with per-partition scale/bias vectors (runtime data,
    no recompile), bf16 out; DVE reduce_sum over keys -> per-row atom sums.
  - The [C -> K] mixing matrix is folded into fp_w1 on host (w1p = A @ fp_w1),
    so the feature_proj MLP runs unchanged: gelu(sum_basis @ w1p) @ fp_w2,
    with PE transposes back to row-major and the host-computed angle/time
    embedding tail added before DMA out ([192, 512] per core).
  - Host (numpy, negligible vs HW): atom fit (cached), angle MLP, sinusoidal
    time MLP, per-core input prep.
"""

import hashlib
import math

import numpy as np

# Problem constants (hardcoded per the task contract).
B, N, K, E = 2, 768, 128, 512
INTER = E // 2
NCORES = 8
RPC = (B * N) // NCORES  # 192 rows per core
C = 32                   # gaussian atoms after compression
PI_REF = 3.14159         # matches reference's gaussian constant
SQ2 = math.sqrt(2.0)
# Derivative_Erf(x) = 2/sqrt(pi) * exp(-x^2); DERF_INV converts back.
DERF_INV = math.sqrt(math.pi) / 2.0

# compat knobs referenced by test.py (unused by this implementation)
USE_FALLBACK_EXP = False
_FUNC_OVERRIDE = None

_COMPILED = {}
_FIT_CACHE = {}


# ---------------- runtime atom fit (numpy-only) ----------------

def _fit_atoms(means, s, xs, rho_row, n_atoms, iters=150, seed_thresh=0.25, seed=0):
    """Fit f_k(d) ~= sum_c A[c,k] exp(-(a_c d + b_c)^2).

    Weighted rows: sqrt(rho) pointwise (variance of the 768-key row sum) plus
    one density-sum row (bias of the row sum). VarPro: A by ridge lstsq each
    step, Levenberg-Marquardt on (a, b)."""
    rng = np.random.default_rng(seed)
    Kn = len(means)
    F = np.exp(-0.5 * ((xs[:, None] - means) / s) ** 2) / (np.sqrt(2 * PI_REF) * s)
    wvar = np.sqrt(rho_row)

    def weight_rows(M):
        return np.concatenate(
            [M * wvar[:, None], (M * rho_row[:, None]).sum(0, keepdims=True)], 0
        )

    Fw = weight_rows(F)
    a = np.empty(n_atoms)
    b = np.empty(n_atoms)
    idx = 0
    for k in np.argsort(s):
        if s[k] < seed_thresh and idx < n_atoms:
            a[idx] = 1.0 / (SQ2 * s[k])
            b[idx] = -means[k] / (SQ2 * s[k])
            idx += 1
    nrem = n_atoms - idx
    if nrem > 0:
        sig_levels = np.geomspace(0.25, 6.0, nrem)
        mus = np.interp(
            np.linspace(0, 1, nrem), np.linspace(0, 1, Kn), np.sort(means)
        )
        rng.shuffle(mus)
        for i in range(nrem):
            a[idx] = 1.0 / sig_levels[i]
            b[idx] = -(mus[i] + rng.normal(0, 0.2)) / sig_levels[i]
            idx += 1

    def atoms_of(a, b):
        Z = a[None, :] * xs[:, None] + b[None, :]
        return Z, np.exp(-(Z ** 2))

    def solve_A(Gw):
        M = Gw.T @ Gw
        M = M + 1e-9 * np.eye(n_atoms) * np.trace(M) / n_atoms
        return np.linalg.solve(M, Gw.T @ Fw)

    lam = 1e-3
    Z, G = atoms_of(a, b)
    Gw = weight_rows(G)
    A = solve_A(Gw)
    R = Fw - Gw @ A
    err = np.linalg.norm(R)
    for _ in range(iters):
        Da = -2 * Z * xs[:, None] * G
        Db = -2 * Z * G
        Daw = weight_rows(Da)
        Dbw = weight_rows(Db)
        AA = A @ A.T
        RA = R @ A.T
        n2 = 2 * n_atoms
        JtJ = np.empty((n2, n2))
        JtJ[:n_atoms, :n_atoms] = AA * (Daw.T @ Daw)
        JtJ[:n_atoms, n_atoms:] = AA * (Daw.T @ Dbw)
        JtJ[n_atoms:, :n_atoms] = JtJ[:n_atoms, n_atoms:].T
        JtJ[n_atoms:, n_atoms:] = AA * (Dbw.T @ Dbw)
        Jtr = np.concatenate([-np.sum(Daw * RA, 0), -np.sum(Dbw * RA, 0)])
        ok = False
        for _ in range(8):
            try:
                step = np.linalg.solve(
                    JtJ + lam * np.diag(np.diag(JtJ)) + 1e-12 * np.eye(n2), -Jtr
                )
            except np.linalg.LinAlgError:
                lam *= 10
                continue
            a2 = a + step[:n_atoms]
            b2 = b + step[n_atoms:]
            Z2, G2 = atoms_of(a2, b2)
            Gw2 = weight_rows(G2)
            A2 = solve_A(Gw2)
            R2 = Fw - Gw2 @ A2
            e2 = np.linalg.norm(R2)
            if e2 < err:
                a, b, Z, G, Gw, A, R, err = a2, b2, Z2, G2, Gw2, A2, R2, e2
                lam = max(lam * 0.5, 1e-7)
                ok = True
                break
            lam *= 4
        if not ok:
            break
    return a, b, A, err / np.linalg.norm(Fw)


def _fit_atoms_best(means, stds, dist_samples):
    """Multi-restart fit keyed on the inputs; returns (a, b, A)."""
    key = hashlib.sha1(
        means.tobytes() + stds.tobytes() + dist_samples.tobytes()
    ).hexdigest()
    if key in _FIT_CACHE:
        return _FIT_CACHE[key]
    s = (np.abs(means * 0) + np.abs(stds) + 0.01).astype(np.float64)
    means64 = means.astype(np.float64)
    xs = np.concatenate([np.arange(0.0, 6.0, 0.01), np.arange(6.0, 40.0, 0.04)])
    hist, _ = np.histogram(dist_samples, bins=np.concatenate([xs, [40.0]]))
    nrows = max(len(dist_samples) // N, 1)
    rho_row = hist.astype(np.float64) / nrows
    best = None
    for trial in range(3):
        a, b, A, rel = _fit_atoms(means64, s, xs, rho_row, C, seed=trial)
        if best is None or rel < best[3]:
            best = (a, b, A, rel)
        if rel < 1.5e-4:
            break
    a, b, A, rel = best
    _FIT_CACHE[key] = (a, b, A)
    return a, b, A


# ---------------- device program ----------------

def _build_nc(use_fallback=None, func_override=None, gelu_override=None):
    import concourse.bass as bass  # noqa: F401
    import concourse.bacc as bacc
    from concourse import mybir
    from concourse.tile import TileContext

    f32 = mybir.dt.float32
    bf16 = mybir.dt.bfloat16
    AF = mybir.ActivationFunctionType
    X = mybir.AxisListType.X

    derf_func = AF.Derivative_Erf if func_override is None else func_override
    gelu_func = AF.Gelu if gelu_override is None else gelu_override

    nc = bacc.Bacc("TRN2", target_bir_lowering=False)

    l5k = nc.dram_tensor("l5k", [5, N], f32, kind="ExternalInput")
    r5r = nc.dram_tensor("r5r", [5, RPC], f32, kind="ExternalInput")
    scl = nc.dram_tensor("scl", [128, C], f32, kind="ExternalInput")
    bia = nc.dram_tensor("bia", [128, C], f32, kind="ExternalInput")
    sel = nc.dram_tensor("sel", [128, C * C], f32, kind="ExternalInput")
    w1p = nc.dram_tensor("w1p", [C, K], f32, kind="ExternalInput")
    w2 = nc.dram_tensor("w2", [K, INTER], f32, kind="ExternalInput")
    ident = nc.dram_tensor("ident", [128, 128], f32, kind="ExternalInput")
    rest = nc.dram_tensor("rest", [RPC, E], f32, kind="ExternalInput")
    out = nc.dram_tensor("out", [RPC, E], f32, kind="ExternalOutput")

    NT = 6 * RPC  # 1152 = 6 key-chunks x 192 rows (keys on partitions)

    with TileContext(nc) as tc:
        with tc.tile_pool(name="sb", bufs=1) as sb:
            # ---- loads needed before phase A/B ----
            l5k_sb = sb.tile([5, N], f32, tag="l5k")
            nc.sync.dma_start(out=l5k_sb, in_=l5k[:, :])
            r5r_sb = sb.tile([5, RPC], f32, tag="r5r")
            nc.gpsimd.dma_start(out=r5r_sb, in_=r5r[:, :])
            scl_sb = sb.tile([128, C], f32, tag="scl")
            nc.scalar.dma_start(out=scl_sb, in_=scl[:, :])
            bia_sb = sb.tile([128, C], f32, tag="bia")
            nc.scalar.dma_start(out=bia_sb, in_=bia[:, :])

            d_all = sb.tile([128, 6, RPC], f32, tag="d_all")
            sbm = sb.tile([C, RPC], f32, tag="sbm")
            scratch = sb.tile([1, 1], f32, tag="scratch")
            sel_f = sb.tile([128, C * C], f32, tag="sel_f")
            nc.gpsimd.dma_start(out=sel_f, in_=sel[:, :])

            tc.strict_bb_all_engine_barrier()

            # bf16 atom-indicator stationaries (exact 1.0/0.0 in bf16)
            sel_sb = sb.tile([128, C * C], bf16, tag="sel")
            nc.vector.tensor_copy(sel_sb, sel_f)

            # ---- phase-C inputs: loaded during phases A/B ----
            w1p_sb = sb.tile([C, K], f32, tag="w1p")
            nc.sync.dma_start(out=w1p_sb, in_=w1p[:, :])
            w2_sb = sb.tile([K, INTER], f32, tag="w2")
            nc.sync.dma_start(out=w2_sb, in_=w2[:, :])
            id_sb = sb.tile([128, 128], f32, tag="id")
            nc.sync.dma_start(out=id_sb, in_=ident[:, :])
            out_sbs = []
            for t in range(2):
                o = sb.tile([96, E], f32, tag=f"out{t}")
                nc.sync.dma_start(out=o, in_=rest[96 * t : 96 * (t + 1), :])
                out_sbs.append(o)

            # preload the Sqrt table while the PE runs the gram matmuls
            nc.scalar.sqrt(scratch, l5k_sb[0:1, 0:1])

            # ---- phase A: pairwise distances, keys on partitions ----
            with tc.tile_pool(name="psA", bufs=1, space="PSUM") as psA:
                # padded free dim keeps each chunk's 768B inside one psum bank
                ps_d = psA.tile([128, 6, 256], f32, tag="d2")
                dview = ps_d[:, :, 0:RPC]
                for gch in range(6):
                    nc.tensor.matmul(
                        ps_d[:, gch, 0:RPC],
                        l5k_sb[:, 128 * gch : 128 * (gch + 1)],
                        r5r_sb,
                        start=True, stop=True,
                    )
                # clamp f32 round-off (gram form can go slightly negative)
                nc.vector.tensor_scalar_max(dview, dview, 0.0)
                nc.scalar.sqrt(d_all, dview)

            # ---- phase B: atom ACT passes; PE reduces over keys into
            # partition c of one accumulating psum tile ----
            with tc.tile_pool(name="psB", bufs=1, space="PSUM") as psB:
                ps_S = psB.tile([C, RPC], f32, tag="S")
                for c in range(C):
                    g = sb.tile([128, 6, RPC], bf16, tag="g", bufs=4)
                    nc.scalar.activation(
                        out=g, in_=d_all, func=derf_func,
                        bias=bia_sb[:, c : c + 1], scale=scl_sb[:, c : c + 1],
                    )
                    for gch in range(6):
                        nc.tensor.matmul(
                            ps_S, sel_sb[:, C * c : C * (c + 1)], g[:, gch, :],
                            start=(c == 0 and gch == 0),
                            stop=(c == C - 1 and gch == 5),
                        )
                nc.vector.tensor_copy(sbm, ps_S)

            # ---- phase C: MLP + transpose + tail add + store ----
            with tc.tile_pool(name="psC", bufs=1, space="PSUM") as psC:
                ps_h = psC.tile([K, RPC], f32, tag="h")
                nc.tensor.matmul(ps_h, w1p_sb, sbm, start=True, stop=True)
                h_sb = sb.tile([K, RPC], f32, tag="h_sb")
                nc.scalar.activation(h_sb, ps_h, gelu_func)

                o_sb = sb.tile([128, 2, RPC], f32, tag="o_sb")
                for e in range(2):
                    ps_o = psC.tile([128, RPC], f32, tag="o", bufs=2)
                    nc.tensor.matmul(
                        ps_o, w2_sb[:, 128 * e : 128 * (e + 1)], h_sb,
                        start=True, stop=True,
                    )
                    nc.vector.tensor_copy(o_sb[:, e, :], ps_o)
                for t in range(2):
                    out_sb = out_sbs[t]
                    for e in range(2):
                        ps_t = psC.tile([96, 128], f32, tag="tr", bufs=2)
                        nc.tensor.transpose(
                            ps_t, o_sb[:, e, 96 * t : 96 * (t + 1)], id_sb
                        )
                        nc.vector.tensor_add(
                            out_sb[:, 128 * e : 128 * (e + 1)],
                            out_sb[:, 128 * e : 128 * (e + 1)],
                            ps_t,
                        )
                    nc.sync.dma_start(
                        out=out[96 * t : 96 * (t + 1), :], in_=out_sb
                    )

    nc.compile()
    return nc


# ---------------- host-side reference tails (numpy, f32) ----------------

def _erf_np(x):
    try:
        from scipy.special import erf
        return erf(x).astype(np.float32)
    except ImportError:
        f = np.frompyfunc(math.erf, 1, 1)
        return f(x.astype(np.float64)).astype(np.float32)


def _gelu_np(x):
    x = x.astype(np.float32)
    return (x * 0.5 * (1.0 + _erf_np(x / np.float32(math.sqrt(2.0))))).astype(
        np.float32
    )


def _silu_np(x):
    x = x.astype(np.float32)
    return (x / (1.0 + np.exp(-x))).astype(np.float32)


def _timestep_emb_np(t, dim):
    half = dim // 2
    freqs = np.exp(
        -np.log(10000.0) * np.arange(half, dtype=np.float32) / np.float32(half)
    ).astype(np.float32)
    a = t.astype(np.float32)[:, None] * freqs[None, :]
    return np.concatenate([np.sin(a), np.cos(a)], axis=-1).astype(np.float32)


def _host_tails(angle, mask_pos, time_pos, ang_w1, ang_w2, t_w1, t_b1, t_w2, t_b2):
    """rest[b, n, :] with rest[..., :INTER] = time_emb[..., :INTER] and
    rest[..., INTER:] = ang_f + time_emb[..., INTER:]."""
    angle = np.asarray(angle, np.float32)
    ang = np.where(np.isposinf(angle), np.float32(0.0), angle).astype(np.float32)
    ang_f = _gelu_np(ang @ np.asarray(ang_w1, np.float32)) @ np.asarray(
        ang_w2, np.float32
    )  # [B, N, INTER]

    def time_mlp(t):
        e = _timestep_emb_np(t, E)
        h = _silu_np(e @ np.asarray(t_w1, np.float32) + np.asarray(t_b1, np.float32))
        return (h @ np.asarray(t_w2, np.float32) + np.asarray(t_b2, np.float32)).astype(
            np.float32
        )

    tp = np.asarray(time_pos)
    te = time_mlp(tp)[:, None, :]                 # [B, 1, E]
    t0e = time_mlp(np.zeros_like(tp))[:, None, :]
    mask = np.asarray(mask_pos, bool)             # [B, N, 1]
    time_emb = np.where(mask, te, t0e).astype(np.float32)  # [B, N, E]

    rest = time_emb.copy()
    rest[..., INTER:] += ang_f.astype(np.float32)
    return rest.astype(np.float32)


def _prep_in_maps(pos, angle, padding_mask, mask_pos, time_pos,
                  means, stds, fp_w1, fp_w2, ang_w1, ang_w2,
                  t_w1, t_b1, t_w2, t_b2, use_fallback=None):
    pos = np.asarray(pos, np.float32)
    pad = np.asarray(padding_mask, bool)
    means = np.asarray(means, np.float32)
    stds = np.asarray(stds, np.float32)

    # distance samples for the fit density (valid keys only)
    pos64 = pos.astype(np.float64)
    d_samples = []
    for bb in range(B):
        dd = np.sqrt(
            np.maximum(
                ((pos64[bb][:, None, :] - pos64[bb][None, :, :]) ** 2).sum(-1), 0.0
            )
        )
        valid = ~pad[bb]
        d_samples.append(dd[:, valid].reshape(-1))
    d_samples = np.concatenate(d_samples)
    a_c, b_c, A = _fit_atoms_best(means, stds, d_samples)

    # fold Derivative_Erf's 2/sqrt(pi) and the mixing into fp_w1
    A_eff = (A * DERF_INV).astype(np.float64)  # [C, K]
    w1p_v = (A_eff @ np.asarray(fp_w1, np.float64)).astype(np.float32)  # [C, K]
    w2_v = np.asarray(fp_w2, np.float32)

    scl_v = np.broadcast_to(a_c.astype(np.float32), (128, C)).copy()
    bia_v = np.broadcast_to(b_c.astype(np.float32), (128, C)).copy()
    # atom-indicator stationaries: slice c is [128, C] with column c all-ones
    sel_v = np.zeros((C, C), np.float32)
    np.fill_diagonal(sel_v, 1.0)
    sel_v = np.broadcast_to(sel_v.reshape(1, C * C), (128, C * C)).copy()
    ident = np.eye(128, dtype=np.float32)

    rest = _host_tails(
        angle, mask_pos, time_pos, ang_w1, ang_w2, t_w1, t_b1, t_w2, t_b2
    )

    in_maps = []
    for core in range(NCORES):
        bb = core // (NCORES // B)
        r0 = (core % (NCORES // B)) * RPC
        p = pos[bb]  # [N, 3]
        n_all = (p.astype(np.float64) ** 2).sum(-1).astype(np.float32)  # [N]
        # stationary: keys (x,y,z,1,n) -> out partition = key
        l5k_v = np.empty((5, N), np.float32)
        l5k_v[0:3] = p.T
        l5k_v[3] = 1.0
        l5k_v[4] = n_all
        if pad[bb].any():
            l5k_v[4, pad[bb]] += 1.0e12  # huge d^2 -> atoms vanish
        # moving: query rows (-2x,-2y,-2z,n,1)
        rows = p[r0 : r0 + RPC]  # [192, 3]
        nr = n_all[r0 : r0 + RPC]
        r5r_v = np.empty((5, RPC), np.float32)
        r5r_v[0:3] = -2.0 * rows.T
        r5r_v[3] = nr
        r5r_v[4] = 1.0
        in_maps.append(
            {
                "l5k": l5k_v,
                "r5r": r5r_v,
                "scl": scl_v,
                "bia": bia_v,
                "sel": sel_v,
                "w1p": w1p_v,
                "w2": w2_v,
                "ident": ident,
                "rest": np.ascontiguousarray(rest[bb, r0 : r0 + RPC, :], np.float32),
            }
        )
    return in_maps


def kernel(pos, angle, node_type_edge, padding_mask, mask_aa, mask_pos, time_pos,
           means, stds, fp_w1, fp_w2, ang_w1, ang_w2, t_w1, t_b1, t_w2, t_b2):
    from concourse.bass_utils import run_bass_kernel_spmd

    key = ("nc", USE_FALLBACK_EXP, _FUNC_OVERRIDE)
    if key not in _COMPILED:
        _COMPILED[key] = _build_nc(func_override=_FUNC_OVERRIDE)
    nc = _COMPILED[key]

    in_maps = _prep_in_maps(
        pos, angle, padding_mask, mask_pos, time_pos, means, stds,
        fp_w1, fp_w2, ang_w1, ang_w2, t_w1, t_b1, t_w2, t_b2,
    )
    res = run_bass_kernel_spmd(nc, in_maps, core_ids=list(range(NCORES)))
    outs = [np.asarray(res.results[c]["out"], np.float32) for c in range(NCORES)]
    full = np.concatenate(outs, axis=0).reshape(B, N, E)
    return full
